# revision 15
# baseline (speedup 1.0000x reference)
"""MoE feed-forward block (B=2, T=2048, D=1024, FF=4096, E=8, top-2) on 8 trn2 cores.

Strategy (expert-parallel, matching the sharding hint):
  - Router (x @ Wr.T, top-2, softmax) computed on host in fp64: it is tiny
    and its output is *indices* + weights, i.e. the dispatch.
  - Dispatch: tokens are gathered per expert on host (the all-to-all), padded
    to a common capacity C, and each of the 8 cores runs the FFN of one
    expert over its routed tokens.
  - Combine: host does out[idx_e] += w_e * y_e (fp32), the weighted
    scatter-add, then reshapes to [B, T, D].

Device kernel: both GEMMs run on the PE in fp8 (e4m3) DoubleRow perf mode,
which contracts K=256 per instruction at 0.5 cycles/row -- 4x the fp16 MAC
rate. Plain e4m3 quantization would cost ~5% relative error, so each GEMM
uses a compensated 3-product split: for operands A (weights) and B
(activations), A = A_hi + A_lo and B = B_hi + B_lo with hi/lo both e4m3 at a
shared power-of-2 scale (lo = quantized residual), and

    A @ B  ~=  A_hi @ B_hi + A_lo @ B_hi + A_hi @ B_lo      (drop lo@lo)

accumulated in one PSUM group. Effective operand precision ~2^-11 (rel err
~7e-4 per operand, ~1.9e-3 end to end), at 0.75x the fp16-matmul cycle
count => ~1.33x PE speedup over the best fp16 kernel.

Layouts (pair dim = the DoubleRow K-pair, i.e. k-blocks 2j/2j+1):
  GEMM1 (h = gelu(x @ W1)): psum[f128, ctile] += W1p[j][128,2,f128].T xp[j]
    [128,2,ctile]; 4 j-tiles x 3 products = 12 matmuls per psum group.
    ACT does gelu twice (e4m3 h_hi straight from PSUM, and fp16 h16);
    DVE forms h_lo = h16 - h_hi (e4m3). W1 scaled by 1024 on host;
    descaled by the ACT `scale` operand.
  GEMM2 (y = h @ W2): psum[c128, dtile] += hp[j2][128,2,c128].T W2p[j2]
    [128,2,dtile]; 16 j2-tiles x 3 products = 48 matmuls per group. The
    h pair AP slices a [128, 32, C] SBUF tile (pair stride = C). W2 scaled
    by 2048 on host; descale folded into the host-side combine weights.
  The C%128 token remainder runs transposed (W2 stationary, h moving, out
  [d128, R]) so its matmul cost scales with R, not with a padded 128.

DMA: x + W1 stream on the SP queue in GEMM1 consumption order; W2 loads ride
the Pool-engine queue so they never delay W1; y stores go back on SP. A
j-outer warmup phase (6 psum banks wide) lets the PE start as soon as the
first x/W1 k-tiles land instead of waiting for the whole first f-quarter.
"""

import sys

sys.path.insert(0, "/opt/trn_rl_repo")

import math
from contextlib import ExitStack

import numpy as np
import ml_dtypes

import concourse.bass as bass
import concourse.tile as tile
from concourse import bacc, mybir
from concourse.bass_utils import run_bass_kernel_spmd

B, T, D, FF, E, TOPK = 2, 2048, 1024, 4096, 8, 2
N_CORES = 8
FC = FF // 128  # 32 f-blocks
KJ1 = D // 256  # 4 K-pair tiles in GEMM1
KJ2 = FF // 256  # 16 K-pair tiles in GEMM2
NQ = 4  # f-quarters of W1 (1024 f-cols each)
S_W1 = 1024.0  # host scale on W1 (power of 2: exact)
S_W2 = 2048.0  # host scale on W2

E4NP = ml_dtypes.float8_e4m3

_cache: dict[int, object] = {}


def _c_chunks(C: int) -> list[tuple[int, int]]:
    """Split C into <=512-wide moving chunks."""
    out, off = [], 0
    while off < C:
        n = min(512, C - off)
        out.append((off, n))
        off += n
    return out


def _build(C: int):
    f16 = mybir.dt.float16
    f32 = mybir.dt.float32
    e4 = mybir.dt.float8e4
    CB, R = C // 128, C % 128

    nc = bacc.Bacc("TRN2", target_bir_lowering=False, debug=False)
    # x pairs: [j, p, i, c] = x[c, (2j+i)*128+p] (hi and lo e4m3 parts)
    xh = nc.dram_tensor("xh", [KJ1, 128, 2, C], e4, kind="ExternalInput").ap()
    xl = nc.dram_tensor("xl", [KJ1, 128, 2, C], e4, kind="ExternalInput").ap()
    # W1 pairs, f-half-quarter-major (512 f-cols per tile for fine-grained
    # streaming): [(hq*4+j), p, i, f'] = 1024*W1[(2j+i)*128+p, hq*512+f']
    w1h = nc.dram_tensor("w1h", [8 * KJ1, 128, 2, 512], e4, kind="ExternalInput").ap()
    w1l = nc.dram_tensor("w1l", [8 * KJ1, 128, 2, 512], e4, kind="ExternalInput").ap()
    # W2 pairs: [j2, p, i, d] = 2048*W2[(2j2+i)*128+p, d]
    w2h = nc.dram_tensor("w2h", [KJ2, 128, 2, 1024], e4, kind="ExternalInput").ap()
    w2l = nc.dram_tensor("w2l", [KJ2, 128, 2, 1024], e4, kind="ExternalInput").ap()
    y = nc.dram_tensor("y", [max(CB, 1) * 128, 1024], f16, kind="ExternalOutput").ap()
    yr = None
    if R:
        # token remainder, d-major: yr[db, p, r] = y_tok[CB*128+r, db*128+p]
        yr = nc.dram_tensor("yr", [8, 128, R], f16, kind="ExternalOutput").ap()

    with tile.TileContext(nc) as tc:
        _emit(nc, tc, xh, xl, w1h, w1l, w2h, w2l, y, yr, C)
    nc.compile()
    return nc


def _emit(nc, tc, xh, xl, w1h, w1l, w2h, w2l, y, yr, C):
    f16 = mybir.dt.float16
    f32 = mybir.dt.float32
    e4 = mybir.dt.float8e4
    GELU = mybir.ActivationFunctionType.Gelu
    CB, R = C // 128, C % 128
    chunks = _c_chunks(C)

    with ExitStack() as ctx:
        xp = ctx.enter_context(tc.tile_pool(name="xp", bufs=1))
        # a half-quarter (hi+lo x 4 k-tiles) must be live at once (8 tiles);
        # 20 bufs gives ~1.5 half-quarters of prefetch
        w1p = ctx.enter_context(tc.tile_pool(name="w1p", bufs=20))
        w2p = ctx.enter_context(tc.tile_pool(name="w2p", bufs=1))
        hp = ctx.enter_context(tc.tile_pool(name="hp", bufs=1))
        h16p = ctx.enter_context(tc.tile_pool(name="h16p", bufs=4))
        ps1p = ctx.enter_context(tc.tile_pool(name="ps1p", bufs=6, space="PSUM"))
        ps2p = ctx.enter_context(tc.tile_pool(name="ps2p", bufs=2, space="PSUM"))
        yp = ctx.enter_context(tc.tile_pool(name="yp", bufs=3))

        # --- input DMA: W1 streams alone on the SP queue in consumption
        # order; x then W2 ride the ACT hwdge queue (seq-only cost there).
        xh_t, xl_t = [], []
        w1_t = {}

        def w1_load(hq, j):
            th = w1p.tile([128, 2, 512], e4, tag="w1", name=f"w1h_{hq}_{j}")
            nc.sync.dma_start(th[:], w1h[hq * KJ1 + j])
            tl = w1p.tile([128, 2, 512], e4, tag="w1", name=f"w1l_{hq}_{j}")
            nc.sync.dma_start(tl[:], w1l[hq * KJ1 + j])
            w1_t[hq, j] = (th, tl)

        for j in range(KJ1):
            txh = xp.tile([128, 2, C], e4, name=f"xh{j}")
            nc.scalar.dma_start(txh[:], xh[j])
            txl = xp.tile([128, 2, C], e4, name=f"xl{j}")
            nc.scalar.dma_start(txl[:], xl[j])
            xh_t.append(txh)
            xl_t.append(txl)
        for hq in range(8):
            for j in range(KJ1):
                w1_load(hq, j)

        w2_t = []
        for j2 in range(KJ2):
            th = w2p.tile([128, 2, 1024], e4, name=f"w2h{j2}")
            nc.scalar.dma_start(th[:], w2h[j2])
            tl = w2p.tile([128, 2, 1024], e4, name=f"w2l{j2}")
            nc.scalar.dma_start(tl[:], w2l[j2])
            w2_t.append((th, tl))

        hh = hp.tile([128, FC, C], e4, name="hh")
        hl = hp.tile([128, FC, C], e4, name="hl")

        def g1_products(ps, fb, coff, clen, j, first, last):
            hq, fbl = fb // 4, fb % 4
            th, tl = w1_t[hq, j]
            lh = th[:, :, fbl * 128 : (fbl + 1) * 128]
            ll = tl[:, :, fbl * 128 : (fbl + 1) * 128]
            rh = xh_t[j][:, :, coff : coff + clen]
            rl = xl_t[j][:, :, coff : coff + clen]
            o = ps[:, :clen]
            DR = mybir.MatmulPerfMode.DoubleRow
            nc.tensor.matmul(o, lh, rh, start=first, stop=False, perf_mode=DR)
            nc.tensor.matmul(o, ll, rh, start=False, stop=False, perf_mode=DR)
            nc.tensor.matmul(o, lh, rl, start=False, stop=last, perf_mode=DR)

        def g1_post(ps, fb, coff, clen):
            # one ACT gelu pass (fp16); Pool casts the hi part to e4m3;
            # DVE forms the residual. Spreads the work over three engines.
            h16 = h16p.tile([128, 512], f16, tag="h16", name=f"h16_{fb}_{coff}")
            nc.scalar.activation(h16[:, :clen], ps[:, :clen], GELU, scale=1.0 / S_W1)
            nc.gpsimd.tensor_copy(hh[:, fb, coff : coff + clen], h16[:, :clen])
            nc.vector.tensor_sub(
                hl[:, fb, coff : coff + clen], h16[:, :clen], hh[:, fb, coff : coff + clen]
            )

        # --- GEMM1. Warmup: j-outer over the 4 f-blocks of half-quarter 0,
        # chunk 0, so the PE starts on (x[0], W1[hq0,j0]) as soon as those
        # land. Then the remaining groups fb-major (matches W1 stream order).
        warm_fb = 4
        coff0, clen0 = chunks[0]
        ps_head = [
            ps1p.tile([128, 512], f32, tag="ps1", name=f"psh_{fb}")
            for fb in range(warm_fb)
        ]
        for j in range(KJ1):
            for fb in range(warm_fb):
                g1_products(
                    ps_head[fb], fb, coff0, clen0, j,
                    first=(j == 0), last=(j == KJ1 - 1),
                )
        for fb in range(warm_fb):
            g1_post(ps_head[fb], fb, coff0, clen0)

        # remainder chunk (cc2) first within each fb, so the GEMM2 remainder
        # phase (which needs cc2 of every fb) unblocks before GEMM1 ends.
        reordered = chunks[2:] + chunks[:2] if len(chunks) > 2 else chunks
        for fb in range(FC):
            for coff, clen in reordered:
                if fb < warm_fb and coff == coff0:
                    continue
                ps = ps1p.tile([128, 512], f32, tag="ps1", name=f"ps1_{fb}_{coff}")
                for j in range(KJ1):
                    g1_products(
                        ps, fb, coff, clen, j,
                        first=(j == 0), last=(j == KJ1 - 1),
                    )
                g1_post(ps, fb, coff, clen)

        DR = mybir.MatmulPerfMode.DoubleRow
        # --- token remainder first (its h chunk is the last thing GEMM1
        # produces, and its small stores must not form the kernel tail):
        # transposed GEMM2, W2 stationary, h moving, out [d-block 128, R].
        if R:
            co = CB * 128
            for db in range(8):
                ps = ps2p.tile([128, 512], f32, tag="ps2", name=f"psr_{db}")
                o = ps[:, :R]
                for j2 in range(KJ2):
                    th, tl = w2_t[j2]
                    lh = th[:, :, db * 128 : (db + 1) * 128]
                    ll = tl[:, :, db * 128 : (db + 1) * 128]
                    rh = hh[:, 2 * j2 : 2 * j2 + 2, co : co + R]
                    rl = hl[:, 2 * j2 : 2 * j2 + 2, co : co + R]
                    nc.tensor.matmul(o, lh, rh, start=(j2 == 0), stop=False, perf_mode=DR)
                    nc.tensor.matmul(o, ll, rh, start=False, stop=False, perf_mode=DR)
                    nc.tensor.matmul(o, lh, rl, start=False, stop=(j2 == KJ2 - 1), perf_mode=DR)
                yrs = yp.tile([128, R], f16, tag="yr", name=f"yr_{db}", bufs=2)
                nc.vector.tensor_copy(yrs[:], ps[:, :R])
                nc.sync.dma_start(yr[db], yrs[:])

        # --- GEMM2: full 128-token blocks, tokens on PSUM partitions.
        for cb in range(CB):
            for doff in (0, 512):
                ps = ps2p.tile([128, 512], f32, tag="ps2", name=f"ps2_{cb}_{doff}")
                for j2 in range(KJ2):
                    th, tl = w2_t[j2]
                    lh = hh[:, 2 * j2 : 2 * j2 + 2, cb * 128 : (cb + 1) * 128]
                    ll = hl[:, 2 * j2 : 2 * j2 + 2, cb * 128 : (cb + 1) * 128]
                    rh = th[:, :, doff : doff + 512]
                    rl = tl[:, :, doff : doff + 512]
                    nc.tensor.matmul(ps[:], lh, rh, start=(j2 == 0), stop=False, perf_mode=DR)
                    nc.tensor.matmul(ps[:], ll, rh, start=False, stop=False, perf_mode=DR)
                    nc.tensor.matmul(ps[:], lh, rl, start=False, stop=(j2 == KJ2 - 1), perf_mode=DR)
                last = cb == CB - 1 and doff == 512
                if not last:
                    ysb = yp.tile([128, 512], f16, tag="y", name=f"y_{cb}_{doff}")
                    nc.vector.tensor_copy(ysb[:], ps[:])
                    nc.sync.dma_start(y[cb * 128 : (cb + 1) * 128, doff : doff + 512], ysb[:])
                else:
                    # split the final store so the copy->DGE->DMA->sem tail
                    # chain runs on a quarter tile, not a full one
                    for so in (0, 256, 384):
                        sl = 256 if so == 0 else 128
                        ysb = yp.tile([128, 512], f16, tag="y", name=f"y_{cb}_{doff}_{so}")
                        nc.vector.tensor_copy(ysb[:, :sl], ps[:, so : so + sl])
                        nc.sync.dma_start(
                            y[cb * 128 : (cb + 1) * 128, doff + so : doff + so + sl],
                            ysb[:, :sl],
                        )


def _route(xf: np.ndarray, Wr: np.ndarray):
    """Host router: top-2 + softmax, fp64 logits for stable decisions."""
    logits = xf.astype(np.float64) @ Wr.astype(np.float64).T  # [N, E]
    top2 = np.argsort(-logits, axis=1, kind="stable")[:, :TOPK]  # [N, 2] desc
    lv = np.take_along_axis(logits, top2, axis=1).astype(np.float32)
    m = lv.max(axis=1, keepdims=True)
    ex = np.exp(lv - m)
    w = (ex / ex.sum(axis=1, keepdims=True)).astype(np.float32)  # [N, 2]
    return top2, w


def _split8(a: np.ndarray, scale: float):
    """hi/lo e4m3 split at a shared (power-of-2) scale."""
    s = (a * scale).astype(np.float32)
    hi = s.astype(E4NP)
    lo = (s - hi.astype(np.float32)).astype(E4NP)
    return hi, lo


def _pack_x(x8: np.ndarray, idx: np.ndarray, C: int) -> np.ndarray:
    """[C_e, 1024] e4m3 rows -> [KJ1, 128, 2, C] pair layout."""
    a = np.zeros((C, D), dtype=E4NP)
    a[: len(idx)] = x8[idx]
    # d = j*256 + i*128 + p
    return np.ascontiguousarray(a.T.reshape(KJ1, 2, 128, C).transpose(0, 2, 1, 3))


def _pack_w1(w: np.ndarray) -> np.ndarray:
    """[1024, 4096] e4m3 -> [8*KJ1, 128, 2, 512] (f-half-quarter-major pairs)."""
    a = w.reshape(KJ1, 2, 128, 8, 512).transpose(3, 0, 2, 1, 4)
    return np.ascontiguousarray(a.reshape(8 * KJ1, 128, 2, 512))


def _pack_w2(w: np.ndarray) -> np.ndarray:
    """[4096, 1024] e4m3 -> [KJ2, 128, 2, 1024] pair layout."""
    return np.ascontiguousarray(w.reshape(KJ2, 2, 128, 1024).transpose(0, 2, 1, 3))


# SBUF budget: h tiles are 64*C B/partition + ~100KB fixed; C <= ~1300 fits.
C_SBUF_MAX = 1296


def _run_pass(x8h, x8l, W1p, W2p, idx, wts, out, trace):
    """One SPMD dispatch over the given per-expert token lists."""
    cmax = max((len(t) for t in idx), default=0)
    C = max(256, ((cmax + 15) // 16) * 16)

    if C not in _cache:
        _cache[C] = _build(C)
    nc = _cache[C]
    CB, R = C // 128, C % 128

    in_maps = []
    for e in range(E):
        in_maps.append(
            {
                "xh": _pack_x(x8h, idx[e], C),
                "xl": _pack_x(x8l, idx[e], C),
                "w1h": W1p[e][0],
                "w1l": W1p[e][1],
                "w2h": W2p[e][0],
                "w2l": W2p[e][1],
            }
        )

    res = run_bass_kernel_spmd(nc, in_maps, list(range(N_CORES)), trace=trace)

    for e in range(E):
        ne = len(idx[e])
        ye = res.results[e]["y"].astype(np.float32)  # [CB*128, 1024]
        if R:
            yre = res.results[e]["yr"].astype(np.float32)  # [8, 128, R]
            ye = np.concatenate([ye, yre.transpose(2, 0, 1).reshape(R, 1024)], axis=0)
        out[idx[e]] += (wts[e] / S_W2)[:, None] * ye[:ne]
    return res


def _run(x, Wr, W1, W2, trace=False):
    xf = np.asarray(x, dtype=np.float32).reshape(-1, D)
    N = xf.shape[0]
    top2, tw = _route(xf, np.asarray(Wr, dtype=np.float32))

    # host-side quantization (scales are powers of 2 -> exact descale)
    x8h, x8l = _split8(xf, 1.0)
    W1p, W2p = [], []
    for e in range(E):
        h1, l1 = _split8(np.asarray(W1[e], np.float32), S_W1)
        W1p.append((_pack_w1(h1), _pack_w1(l1)))
        h2, l2 = _split8(np.asarray(W2[e], np.float32), S_W2)
        W2p.append((_pack_w2(h2), _pack_w2(l2)))

    idx, wts = [], []
    for e in range(E):
        mask = top2 == e  # [N, 2]
        tok = np.nonzero(mask.any(axis=1))[0]
        k = np.argmax(mask[tok], axis=1)
        we = tw[tok, k]
        idx.append(tok)
        wts.append(we.astype(np.float32))

    cmax = max(len(t) for t in idx)
    n_pass = max(1, math.ceil(cmax / C_SBUF_MAX))

    out = np.zeros((N, D), dtype=np.float32)
    res = None
    for p in range(n_pass):
        idx_p = [t[p * len(t) // n_pass : (p + 1) * len(t) // n_pass] for t in idx]
        wts_p = [w[p * len(w) // n_pass : (p + 1) * len(w) // n_pass] for w in wts]
        res = _run_pass(x8h, x8l, W1p, W2p, idx_p, wts_p, out, trace)
    return out.reshape(B, T, D), res


def kernel(x, Wr, W1, W2):
    out, _ = _run(x, Wr, W1, W2, trace=False)
    return out


# revision 16
# speedup vs baseline: 1.1542x; 1.1542x over previous
"""MoE feed-forward block (B=2, T=2048, D=1024, FF=4096, E=8, top-2) on 8 trn2 cores.

Strategy (expert-parallel, matching the sharding hint):
  - Router (x @ Wr.T, top-2, softmax) computed on host in fp64: it is tiny
    and its output is *indices* + weights, i.e. the dispatch.
  - Dispatch: tokens are gathered per expert on host (the all-to-all), padded
    to a common capacity C, and each of the 8 cores runs the FFN of one
    expert over its routed tokens.
  - Combine: host does out[idx_e] += w_e * y_e (fp32), the weighted
    scatter-add, then reshapes to [B, T, D].

Device kernel: both GEMMs run on the PE in fp8 (e4m3) DoubleRow perf mode,
which contracts K=256 per instruction at 0.5 cycles/row -- 4x the fp16 MAC
rate. Plain e4m3 quantization would cost ~5% relative error, so each GEMM
uses a compensated 3-product split: for operands A (weights) and B
(activations), A = A_hi + A_lo and B = B_hi + B_lo with hi/lo both e4m3 at a
shared power-of-2 scale (lo = quantized residual), and

    A @ B  ~=  A_hi @ B_hi + A_lo @ B_hi + A_hi @ B_lo      (drop lo@lo)

accumulated in one PSUM group. Effective operand precision ~2^-11 (rel err
~7e-4 per operand, ~1.9e-3 end to end), at 0.75x the fp16-matmul cycle
count => ~1.33x PE speedup over the best fp16 kernel.

Layouts (pair dim = the DoubleRow K-pair, i.e. k-blocks 2j/2j+1):
  GEMM1 (h = gelu(x @ W1)): psum[f128, ctile] += W1p[j][128,2,f128].T xp[j]
    [128,2,ctile]; 4 j-tiles x 3 products = 12 matmuls per psum group.
    ACT does gelu twice (e4m3 h_hi straight from PSUM, and fp16 h16);
    DVE forms h_lo = h16 - h_hi (e4m3). W1 scaled by 1024 on host;
    descaled by the ACT `scale` operand.
  GEMM2 (y = h @ W2): psum[c128, dtile] += hp[j2][128,2,c128].T W2p[j2]
    [128,2,dtile]; 16 j2-tiles x 3 products = 48 matmuls per group. The
    h pair AP slices a [128, 32, C] SBUF tile (pair stride = C). W2 scaled
    by 2048 on host; descale folded into the host-side combine weights.
  The C%128 token remainder runs transposed (W2 stationary, h moving, out
  [d128, R]) so its matmul cost scales with R, not with a padded 128.

DMA: x + W1 stream on the SP queue in GEMM1 consumption order; W2 loads ride
the Pool-engine queue so they never delay W1; y stores go back on SP. A
j-outer warmup phase (6 psum banks wide) lets the PE start as soon as the
first x/W1 k-tiles land instead of waiting for the whole first f-quarter.
"""

import sys

sys.path.insert(0, "/opt/trn_rl_repo")

import math
from contextlib import ExitStack

import numpy as np
import ml_dtypes

import concourse.bass as bass
import concourse.tile as tile
from concourse import bacc, mybir
from concourse.bass_utils import run_bass_kernel_spmd

B, T, D, FF, E, TOPK = 2, 2048, 1024, 4096, 8, 2
N_CORES = 8
FC = FF // 128  # 32 f-blocks
KJ1 = D // 256  # 4 K-pair tiles in GEMM1
KJ2 = FF // 256  # 16 K-pair tiles in GEMM2
NQ = 4  # f-quarters of W1 (1024 f-cols each)
S_W1 = 1024.0  # host scale on W1 (power of 2: exact)
S_W2 = 2048.0  # host scale on W2

E4NP = ml_dtypes.float8_e4m3

_cache: dict[int, object] = {}


def _c_chunks(C: int) -> list[tuple[int, int]]:
    """Split C into <=512-wide moving chunks."""
    out, off = [], 0
    while off < C:
        n = min(512, C - off)
        out.append((off, n))
        off += n
    return out


def _build(C: int):
    f16 = mybir.dt.float16
    f32 = mybir.dt.float32
    e4 = mybir.dt.float8e4
    CB, R = C // 128, C % 128

    nc = bacc.Bacc("TRN2", target_bir_lowering=False, debug=False)
    # x pairs: [j, p, i, c] = x[c, (2j+i)*128+p] (hi and lo e4m3 parts)
    xh = nc.dram_tensor("xh", [KJ1, 128, 2, C], e4, kind="ExternalInput").ap()
    xl = nc.dram_tensor("xl", [KJ1, 128, 2, C], e4, kind="ExternalInput").ap()
    # W1 pairs, f-half-quarter-major (512 f-cols per tile for fine-grained
    # streaming): [(hq*4+j), p, i, f'] = 1024*W1[(2j+i)*128+p, hq*512+f']
    w1h = nc.dram_tensor("w1h", [8 * KJ1, 128, 2, 512], e4, kind="ExternalInput").ap()
    w1l = nc.dram_tensor("w1l", [8 * KJ1, 128, 2, 512], e4, kind="ExternalInput").ap()
    # W2 pairs: [j2, p, i, d] = 2048*W2[(2j2+i)*128+p, d]
    w2h = nc.dram_tensor("w2h", [KJ2, 128, 2, 1024], e4, kind="ExternalInput").ap()
    w2l = nc.dram_tensor("w2l", [KJ2, 128, 2, 1024], e4, kind="ExternalInput").ap()
    y = nc.dram_tensor("y", [max(CB, 1) * 128, 1024], f16, kind="ExternalOutput").ap()
    yr = None
    if R:
        # token remainder, d-major: yr[db, p, r] = y_tok[CB*128+r, db*128+p]
        yr = nc.dram_tensor("yr", [8, 128, R], f16, kind="ExternalOutput").ap()

    with tile.TileContext(nc) as tc:
        _emit(nc, tc, xh, xl, w1h, w1l, w2h, w2l, y, yr, C)
    nc.compile()
    return nc


def _emit(nc, tc, xh, xl, w1h, w1l, w2h, w2l, y, yr, C):
    f16 = mybir.dt.float16
    f32 = mybir.dt.float32
    e4 = mybir.dt.float8e4
    GELU = mybir.ActivationFunctionType.Gelu
    CB, R = C // 128, C % 128
    chunks = _c_chunks(C)

    with ExitStack() as ctx:
        xp = ctx.enter_context(tc.tile_pool(name="xp", bufs=1))
        # a half-quarter (hi+lo x 4 k-tiles) must be live at once (8 tiles);
        # 20 bufs gives ~1.5 half-quarters of prefetch
        w1p = ctx.enter_context(tc.tile_pool(name="w1p", bufs=20))
        w2p = ctx.enter_context(tc.tile_pool(name="w2p", bufs=1))
        hp = ctx.enter_context(tc.tile_pool(name="hp", bufs=1))
        h16p = ctx.enter_context(tc.tile_pool(name="h16p", bufs=4))
        ps1p = ctx.enter_context(tc.tile_pool(name="ps1p", bufs=6, space="PSUM"))
        ps2p = ctx.enter_context(tc.tile_pool(name="ps2p", bufs=2, space="PSUM"))
        yp = ctx.enter_context(tc.tile_pool(name="yp", bufs=3))

        # --- input DMA: W1 streams alone on the SP queue in consumption
        # order; x then W2 ride the ACT hwdge queue (seq-only cost there).
        xh_t, xl_t = [], []
        w1_t = {}

        def w1_load(hq, j):
            th = w1p.tile([128, 2, 512], e4, tag="w1", name=f"w1h_{hq}_{j}")
            nc.sync.dma_start(th[:], w1h[hq * KJ1 + j])
            tl = w1p.tile([128, 2, 512], e4, tag="w1", name=f"w1l_{hq}_{j}")
            nc.sync.dma_start(tl[:], w1l[hq * KJ1 + j])
            w1_t[hq, j] = (th, tl)

        for j in range(KJ1):
            txh = xp.tile([128, 2, C], e4, name=f"xh{j}")
            nc.scalar.dma_start(txh[:], xh[j])
            txl = xp.tile([128, 2, C], e4, name=f"xl{j}")
            nc.scalar.dma_start(txl[:], xl[j])
            xh_t.append(txh)
            xl_t.append(txl)
        for hq in range(8):
            for j in range(KJ1):
                w1_load(hq, j)

        # W2 on SP after W1: SP is otherwise idle until the y stores, and
        # the data still lands well before GEMM2 begins.
        w2_t = []
        for j2 in range(KJ2):
            th = w2p.tile([128, 2, 1024], e4, name=f"w2h{j2}")
            nc.sync.dma_start(th[:], w2h[j2])
            tl = w2p.tile([128, 2, 1024], e4, name=f"w2l{j2}")
            nc.sync.dma_start(tl[:], w2l[j2])
            w2_t.append((th, tl))

        hh = hp.tile([128, FC, C], e4, name="hh")
        hl = hp.tile([128, FC, C], e4, name="hl")

        def g1_products(ps, fb, coff, clen, j, first, last):
            hq, fbl = fb // 4, fb % 4
            th, tl = w1_t[hq, j]
            lh = th[:, :, fbl * 128 : (fbl + 1) * 128]
            ll = tl[:, :, fbl * 128 : (fbl + 1) * 128]
            rh = xh_t[j][:, :, coff : coff + clen]
            rl = xl_t[j][:, :, coff : coff + clen]
            o = ps[:, :clen]
            DR = mybir.MatmulPerfMode.DoubleRow
            nc.tensor.matmul(o, lh, rh, start=first, stop=False, perf_mode=DR)
            nc.tensor.matmul(o, ll, rh, start=False, stop=False, perf_mode=DR)
            nc.tensor.matmul(o, lh, rl, start=False, stop=last, perf_mode=DR)

        def g1_post(ps, fb, coff, clen):
            # one ACT gelu pass (fp16); Pool casts the hi part to e4m3;
            # DVE forms the residual. Spreads the work over three engines.
            h16 = h16p.tile([128, 512], f16, tag="h16", name=f"h16_{fb}_{coff}")
            nc.scalar.activation(h16[:, :clen], ps[:, :clen], GELU, scale=1.0 / S_W1)
            nc.gpsimd.tensor_copy(hh[:, fb, coff : coff + clen], h16[:, :clen])
            nc.vector.tensor_sub(
                hl[:, fb, coff : coff + clen], h16[:, :clen], hh[:, fb, coff : coff + clen]
            )

        # --- GEMM1. Warmup: j-outer over the 4 f-blocks of half-quarter 0,
        # chunk 0, so the PE starts on (x[0], W1[hq0,j0]) as soon as those
        # land. Then the remaining groups fb-major (matches W1 stream order).
        warm_fb = 4
        coff0, clen0 = chunks[0]
        ps_head = [
            ps1p.tile([128, 512], f32, tag="ps1", name=f"psh_{fb}")
            for fb in range(warm_fb)
        ]
        for j in range(KJ1):
            for fb in range(warm_fb):
                g1_products(
                    ps_head[fb], fb, coff0, clen0, j,
                    first=(j == 0), last=(j == KJ1 - 1),
                )
        for fb in range(warm_fb):
            g1_post(ps_head[fb], fb, coff0, clen0)

        # remainder chunk (cc2) first within each fb, so the GEMM2 remainder
        # phase (which needs cc2 of every fb) unblocks before GEMM1 ends.
        reordered = chunks[2:] + chunks[:2] if len(chunks) > 2 else chunks
        for fb in range(FC):
            for coff, clen in reordered:
                if fb < warm_fb and coff == coff0:
                    continue
                ps = ps1p.tile([128, 512], f32, tag="ps1", name=f"ps1_{fb}_{coff}")
                for j in range(KJ1):
                    g1_products(
                        ps, fb, coff, clen, j,
                        first=(j == 0), last=(j == KJ1 - 1),
                    )
                g1_post(ps, fb, coff, clen)

        DR = mybir.MatmulPerfMode.DoubleRow
        # --- token remainder first (its h chunk is the last thing GEMM1
        # produces, and its small stores must not form the kernel tail):
        # transposed GEMM2, W2 stationary, h moving, out [d-block 128, R].
        if R:
            co = CB * 128
            for db in range(8):
                ps = ps2p.tile([128, 512], f32, tag="ps2", name=f"psr_{db}")
                o = ps[:, :R]
                for j2 in range(KJ2):
                    th, tl = w2_t[j2]
                    lh = th[:, :, db * 128 : (db + 1) * 128]
                    ll = tl[:, :, db * 128 : (db + 1) * 128]
                    rh = hh[:, 2 * j2 : 2 * j2 + 2, co : co + R]
                    rl = hl[:, 2 * j2 : 2 * j2 + 2, co : co + R]
                    nc.tensor.matmul(o, lh, rh, start=(j2 == 0), stop=False, perf_mode=DR)
                    nc.tensor.matmul(o, ll, rh, start=False, stop=False, perf_mode=DR)
                    nc.tensor.matmul(o, lh, rl, start=False, stop=(j2 == KJ2 - 1), perf_mode=DR)
                yrs = yp.tile([128, R], f16, tag="yr", name=f"yr_{db}", bufs=2)
                nc.vector.tensor_copy(yrs[:], ps[:, :R])
                nc.sync.dma_start(yr[db], yrs[:])

        # --- GEMM2: full 128-token blocks, tokens on PSUM partitions.
        for cb in range(CB):
            for doff in (0, 512):
                ps = ps2p.tile([128, 512], f32, tag="ps2", name=f"ps2_{cb}_{doff}")
                for j2 in range(KJ2):
                    th, tl = w2_t[j2]
                    lh = hh[:, 2 * j2 : 2 * j2 + 2, cb * 128 : (cb + 1) * 128]
                    ll = hl[:, 2 * j2 : 2 * j2 + 2, cb * 128 : (cb + 1) * 128]
                    rh = th[:, :, doff : doff + 512]
                    rl = tl[:, :, doff : doff + 512]
                    nc.tensor.matmul(ps[:], lh, rh, start=(j2 == 0), stop=False, perf_mode=DR)
                    nc.tensor.matmul(ps[:], ll, rh, start=False, stop=False, perf_mode=DR)
                    nc.tensor.matmul(ps[:], lh, rl, start=False, stop=(j2 == KJ2 - 1), perf_mode=DR)
                last = cb == CB - 1 and doff == 512
                if not last:
                    ysb = yp.tile([128, 512], f16, tag="y", name=f"y_{cb}_{doff}")
                    nc.vector.tensor_copy(ysb[:], ps[:])
                    nc.sync.dma_start(y[cb * 128 : (cb + 1) * 128, doff : doff + 512], ysb[:])
                else:
                    # split the final store so the copy->DGE->DMA->sem tail
                    # chain runs on a quarter tile, not a full one
                    for so in (0, 256, 384):
                        sl = 256 if so == 0 else 128
                        ysb = yp.tile([128, 512], f16, tag="y", name=f"y_{cb}_{doff}_{so}")
                        nc.vector.tensor_copy(ysb[:, :sl], ps[:, so : so + sl])
                        nc.sync.dma_start(
                            y[cb * 128 : (cb + 1) * 128, doff + so : doff + so + sl],
                            ysb[:, :sl],
                        )


def _route(xf: np.ndarray, Wr: np.ndarray):
    """Host router: top-2 + softmax, fp64 logits for stable decisions."""
    logits = xf.astype(np.float64) @ Wr.astype(np.float64).T  # [N, E]
    top2 = np.argsort(-logits, axis=1, kind="stable")[:, :TOPK]  # [N, 2] desc
    lv = np.take_along_axis(logits, top2, axis=1).astype(np.float32)
    m = lv.max(axis=1, keepdims=True)
    ex = np.exp(lv - m)
    w = (ex / ex.sum(axis=1, keepdims=True)).astype(np.float32)  # [N, 2]
    return top2, w


def _split8(a: np.ndarray, scale: float):
    """hi/lo e4m3 split at a shared (power-of-2) scale."""
    s = (a * scale).astype(np.float32)
    hi = s.astype(E4NP)
    lo = (s - hi.astype(np.float32)).astype(E4NP)
    return hi, lo


def _pack_x(x8: np.ndarray, idx: np.ndarray, C: int) -> np.ndarray:
    """[C_e, 1024] e4m3 rows -> [KJ1, 128, 2, C] pair layout."""
    a = np.zeros((C, D), dtype=E4NP)
    a[: len(idx)] = x8[idx]
    # d = j*256 + i*128 + p
    return np.ascontiguousarray(a.T.reshape(KJ1, 2, 128, C).transpose(0, 2, 1, 3))


def _pack_w1(w: np.ndarray) -> np.ndarray:
    """[1024, 4096] e4m3 -> [8*KJ1, 128, 2, 512] (f-half-quarter-major pairs)."""
    a = w.reshape(KJ1, 2, 128, 8, 512).transpose(3, 0, 2, 1, 4)
    return np.ascontiguousarray(a.reshape(8 * KJ1, 128, 2, 512))


def _pack_w2(w: np.ndarray) -> np.ndarray:
    """[4096, 1024] e4m3 -> [KJ2, 128, 2, 1024] pair layout."""
    return np.ascontiguousarray(w.reshape(KJ2, 2, 128, 1024).transpose(0, 2, 1, 3))


# SBUF budget: h tiles are 64*C B/partition + ~100KB fixed; C <= ~1300 fits.
C_SBUF_MAX = 1296


def _run_pass(x8h, x8l, W1p, W2p, idx, wts, out, trace):
    """One SPMD dispatch over the given per-expert token lists."""
    cmax = max((len(t) for t in idx), default=0)
    C = max(256, ((cmax + 15) // 16) * 16)

    if C not in _cache:
        _cache[C] = _build(C)
    nc = _cache[C]
    CB, R = C // 128, C % 128

    in_maps = []
    for e in range(E):
        in_maps.append(
            {
                "xh": _pack_x(x8h, idx[e], C),
                "xl": _pack_x(x8l, idx[e], C),
                "w1h": W1p[e][0],
                "w1l": W1p[e][1],
                "w2h": W2p[e][0],
                "w2l": W2p[e][1],
            }
        )

    res = run_bass_kernel_spmd(nc, in_maps, list(range(N_CORES)), trace=trace)

    for e in range(E):
        ne = len(idx[e])
        ye = res.results[e]["y"].astype(np.float32)  # [CB*128, 1024]
        if R:
            yre = res.results[e]["yr"].astype(np.float32)  # [8, 128, R]
            ye = np.concatenate([ye, yre.transpose(2, 0, 1).reshape(R, 1024)], axis=0)
        out[idx[e]] += (wts[e] / S_W2)[:, None] * ye[:ne]
    return res


def _run(x, Wr, W1, W2, trace=False):
    xf = np.asarray(x, dtype=np.float32).reshape(-1, D)
    N = xf.shape[0]
    top2, tw = _route(xf, np.asarray(Wr, dtype=np.float32))

    # host-side quantization (scales are powers of 2 -> exact descale)
    x8h, x8l = _split8(xf, 1.0)
    W1p, W2p = [], []
    for e in range(E):
        h1, l1 = _split8(np.asarray(W1[e], np.float32), S_W1)
        W1p.append((_pack_w1(h1), _pack_w1(l1)))
        h2, l2 = _split8(np.asarray(W2[e], np.float32), S_W2)
        W2p.append((_pack_w2(h2), _pack_w2(l2)))

    idx, wts = [], []
    for e in range(E):
        mask = top2 == e  # [N, 2]
        tok = np.nonzero(mask.any(axis=1))[0]
        k = np.argmax(mask[tok], axis=1)
        we = tw[tok, k]
        idx.append(tok)
        wts.append(we.astype(np.float32))

    cmax = max(len(t) for t in idx)
    n_pass = max(1, math.ceil(cmax / C_SBUF_MAX))

    out = np.zeros((N, D), dtype=np.float32)
    res = None
    for p in range(n_pass):
        idx_p = [t[p * len(t) // n_pass : (p + 1) * len(t) // n_pass] for t in idx]
        wts_p = [w[p * len(w) // n_pass : (p + 1) * len(w) // n_pass] for w in wts]
        res = _run_pass(x8h, x8l, W1p, W2p, idx_p, wts_p, out, trace)
    return out.reshape(B, T, D), res


def kernel(x, Wr, W1, W2):
    out, _ = _run(x, Wr, W1, W2, trace=False)
    return out
